# revision 7
# baseline (speedup 1.0000x reference)
"""MHA kernel for Trainium2, 8 NeuronCores.

Problem: B=4, S=2048, D=512, H=8 heads (head_dim 64).
  Q = x @ Wq.T ; K = x @ Wk.T ; V = x @ Wv.T  (per-head split)
  out = softmax(Q K^T / sqrt(512)) V         (concat heads)

Sharding: 8 cores = 4 batches x 2 head-groups (4 heads each).
Core c handles batch c//2, heads (c%2)*4 .. (c%2)*4+4.
Each core receives x[b] [2048,512] and the 256-row slices of Wq/Wk/Wv
for its heads, and produces y [2048,256] = out[b, :, g*256:(g+1)*256].

Per-core kernel (all compute in SBUF/PSUM, no collectives):
  1. PE-transpose x -> xT [512d, 2048s], W slices -> wT [512d, 256m].
  2. Projections (float32r matmuls): QT/KT [256dq, 2048s] with head
     pairs stacked on partitions (64+64), V in natural [2048s, 256dv]
     layout augmented with a ones column per head (for softmax sums).
  3. Per (head, q-chunk of 512): S^T tiles [128k, 512q] via matmuls
     contracting head_dim=64; exp on ScalarE directly from PSUM in
     multi-bank groups (scale=1/sqrt(512) folded in), bf16 E in SBUF.
     No max-subtraction: |scores/sqrt(512)| < ~1 by construction.
  4. PV: O^T[65, 512q] (64 dims + rowsum) accumulated over 16 k-chunks
     with lhsT=V_aug bf16, rhs=E bf16.
  5. PE-transpose O^T -> [128q, 65], normalize by reciprocal(rowsum)
     on VectorE, assemble y and DMA out.
"""

import os
import sys

import numpy as np

for _p in ("/opt/trn_rl_repo", "/root/.axon_site/_ro/trn_rl_repo"):
    if os.path.isdir(_p) and _p not in sys.path:
        sys.path.append(_p)

import concourse.bass as bass
import concourse.mybir as mybir
import concourse.tile as tile
from concourse import bacc
from concourse.bass_utils import run_bass_kernel_spmd
from concourse.masks import make_identity

F32 = mybir.dt.float32
F32R = mybir.dt.float32r
BF16 = mybir.dt.bfloat16

B, S, D, H = 4, 2048, 512, 8
HD = D // H          # 64
HL = 4               # heads per core
DQ = HL * HD         # 256 output dims per core
P = 128
DJ = D // P          # 4 contraction chunks
NT = S // P          # 16 s-tiles of 128
NQC = S // 512       # 4 q-chunks of 512
SCALE = 1.0 / float(np.sqrt(np.float32(D)))

# kc-groups for S^T psum/exp batching: (start, size) in 128-k-chunks
KC_GROUPS = [(0, 3), (3, 3), (6, 3), (9, 3), (12, 2), (14, 2)]

EXP = mybir.ActivationFunctionType.Exp


def r(ap):
    return ap.bitcast(F32R)


def build_nc():
    nc = bacc.Bacc("TRN2", target_bir_lowering=False, debug=False, num_devices=8)
    x = nc.dram_tensor("x", [S, D], F32, kind="ExternalInput")
    wq = nc.dram_tensor("wq", [DQ, D], F32, kind="ExternalInput")
    wk = nc.dram_tensor("wk", [DQ, D], F32, kind="ExternalInput")
    wv = nc.dram_tensor("wv", [DQ, D], F32, kind="ExternalInput")
    y = nc.dram_tensor("y", [S, DQ], F32, kind="ExternalOutput")

    with tile.TileContext(nc) as tc:
        with (
            tc.tile_pool(name="const", bufs=1) as cp,
            tc.tile_pool(name="xin", bufs=4) as xin,
            tc.tile_pool(name="win", bufs=2) as win,
            tc.tile_pool(name="ot", bufs=2) as otp,
            tc.tile_pool(name="ep", bufs=2) as ep,
            tc.tile_pool(name="pp", bufs=2, space="PSUM") as pp,
            tc.tile_pool(name="pq", bufs=1, space="PSUM") as pq,
        ):
            ident = cp.tile([P, P], F32)
            make_identity(nc, ident)

            # PE warm-up: ~20 dummy matmuls (>3.4us at cold clock) so the
            # HAM frequency governor reaches 2.4GHz before real work; runs
            # concurrently with the input DMAs.
            wu = cp.tile([P, 512], BF16)
            nc.vector.memset(wu[:], 0.0)
            for _ in range(20):
                pwu = pp.tile([P, 512], F32, tag="ps")
                nc.tensor.matmul(
                    pwu[:], lhsT=wu[:, :P], rhs=wu[:], start=True, stop=True
                )

            xT = cp.tile([P, DJ, S], F32R)       # x.T  [d, s]
            wTs = {}
            for name, w in (("q", wq), ("k", wk), ("v", wv)):
                wTs[name] = cp.tile([P, DJ, DQ], F32R, name=f"wT_{name}")  # W.T [d, m]
            QT = cp.tile([P, 2, S], F32R)        # Q.T, head pair per 128 parts
            KT = cp.tile([P, 2, S], F32R)
            Vaug = cp.tile([P, NT, HL * (HD + 1)], BF16)  # V + ones cols
            Ofin = cp.tile([P, NT, DQ], F32)

            # ---- transpose x into xT ----
            for tq in range(4):
                xts = []
                for u in range(4):
                    t = xin.tile([P, D], F32, tag="x")
                    nc.sync.dma_start(t[:], x[(tq * 4 + u) * P : (tq * 4 + u + 1) * P, :])
                    xts.append(t)
                for j in range(DJ):
                    pt = pp.tile([P, 512], F32, tag="ps")
                    for u in range(4):
                        nc.tensor.transpose(
                            pt[:, u * P : (u + 1) * P],
                            xts[u][:, j * P : (j + 1) * P],
                            ident,
                        )
                    nc.vector.tensor_copy(xT[:, j, tq * 512 : (tq + 1) * 512], pt[:])

            # ---- transpose W slices ----
            for name, w in (("q", wq), ("k", wk), ("v", wv)):
                wt0 = win.tile([P, D], F32, tag="w")
                wt1 = win.tile([P, D], F32, tag="w")
                nc.sync.dma_start(wt0[:], w[0:P, :])
                nc.sync.dma_start(wt1[:], w[P : 2 * P, :])
                wts = (wt0, wt1)
                for j in range(DJ):
                    pt = pp.tile([P, 512], F32, tag="ps")
                    for p2 in range(2):
                        nc.tensor.transpose(
                            pt[:, p2 * P : (p2 + 1) * P],
                            wts[p2][:, j * P : (j + 1) * P],
                            ident,
                        )
                    nc.vector.tensor_copy(wTs[name][:, j, :], pt[:, :DQ])

            # ---- projections: QT / KT ----
            for dst, wT in ((QT, wTs["q"]), (KT, wTs["k"])):
                for p2 in range(2):
                    for sc in range(NQC):
                        pt = pp.tile([P, 512], F32, tag="ps")
                        for j in range(DJ):
                            nc.tensor.matmul(
                                pt[:],
                                lhsT=wT[:, j, p2 * P : (p2 + 1) * P],
                                rhs=xT[:, j, sc * 512 : (sc + 1) * 512],
                                start=(j == 0),
                                stop=(j == DJ - 1),
                            )
                        nc.vector.tensor_copy(dst[:, p2, sc * 512 : (sc + 1) * 512], pt[:])

            # ---- V natural + ones columns ----
            nc.vector.memset(Vaug[:], 1.0)
            for t in range(NT):
                pt = pp.tile([P, 512], F32, tag="ps")
                for j in range(DJ):
                    nc.tensor.matmul(
                        pt[:, :DQ],
                        lhsT=xT[:, j, t * P : (t + 1) * P],
                        rhs=wTs["v"][:, j, :],
                        start=(j == 0),
                        stop=(j == DJ - 1),
                    )
                vdst = Vaug[:, t, :].rearrange("p (h c) -> p h c", h=HL)[:, :, :HD]
                vsrc = pt[:, :DQ].rearrange("p (h c) -> p h c", h=HL)
                nc.vector.tensor_copy(vdst, vsrc)

            # ---- attention ----
            # Heads are processed in pairs: head-even lives on partitions
            # 0-63, head-odd on 64-127, so their QK matmuls land on the two
            # independent 64x128 PE row-tiles (T0/T8); interleaving them
            # hides weight-load/drain behind the other tile's streaming.
            def head_block(p2, qc):
                q0, q1 = qc * 512, (qc + 1) * 512
                E0 = ep.tile([P, NT, 512], BF16, tag="E0")
                E1 = ep.tile([P, NT, 512], BF16, tag="E1")
                for g0, gsz in KC_GROUPS:
                    G0 = pq.tile([P, 3, 512], F32, tag="G0")
                    G1 = pq.tile([P, 3, 512], F32, tag="G1")
                    for i in range(gsz):
                        kc = g0 + i
                        for e, G in ((0, G0), (1, G1)):
                            nc.tensor.matmul(
                                G[:, i, :],
                                lhsT=KT[e * HD : (e + 1) * HD, p2, kc * P : (kc + 1) * P],
                                rhs=QT[e * HD : (e + 1) * HD, p2, q0:q1],
                                start=True,
                                stop=True,
                            )
                    for E, G in ((E0, G0), (E1, G1)):
                        nc.scalar.activation(
                            E[:, g0 : g0 + gsz, :], G[:, :gsz, :], EXP, scale=SCALE
                        )
                pos = []
                for e, E in ((0, E0), (1, E1)):
                    hl = p2 * 2 + e
                    po = pp.tile([P, 512], F32, tag="ps", name=f"po_{e}")
                    for kc in range(NT):
                        nc.tensor.matmul(
                            po[: HD + 1, :],
                            lhsT=Vaug[:, kc, hl * (HD + 1) : (hl + 1) * (HD + 1)],
                            rhs=E[:, kc, :],
                            start=(kc == 0),
                            stop=(kc == NT - 1),
                        )
                    pos.append(po)
                for e, po in ((0, pos[0]), (1, pos[1])):
                    hl = p2 * 2 + e
                    ot = otp.tile([HD + 1, 512], F32, tag="ot")
                    nc.vector.tensor_copy(ot[:], po[: HD + 1, :])
                    pt = pp.tile([P, 512], F32, tag="ps", name=f"pt_{e}")
                    for u in range(4):
                        nc.tensor.transpose(
                            pt[:, u * (HD + 1) : (u + 1) * (HD + 1)],
                            ot[:, u * P : (u + 1) * P],
                            ident[: HD + 1, : HD + 1],
                        )
                    rt = otp.tile([P, 4], F32, tag="rt")
                    tv = pt[:, : 4 * (HD + 1)].rearrange("p (u c) -> p u c", u=4)
                    nc.vector.reciprocal(rt[:], tv[:, :, HD])
                    for u in range(4):
                        nc.vector.tensor_scalar_mul(
                            Ofin[:, qc * 4 + u, hl * HD : (hl + 1) * HD],
                            tv[:, u, :HD],
                            rt[:, u : u + 1],
                        )

            for p2 in range(2):
                for qc in range(NQC):
                    head_block(p2, qc)

            nc.sync.dma_start(y[:].rearrange("(t p) c -> p t c", p=P), Ofin[:])

    nc.compile()
    return nc


_NC_CACHE = None


def _get_nc():
    global _NC_CACHE
    if _NC_CACHE is None:
        _NC_CACHE = build_nc()
    return _NC_CACHE


def _in_maps(x, Wq, Wk, Wv):
    x = np.asarray(x, dtype=np.float32)
    Wq = np.asarray(Wq, dtype=np.float32)
    Wk = np.asarray(Wk, dtype=np.float32)
    Wv = np.asarray(Wv, dtype=np.float32)
    maps = []
    for c in range(8):
        b, g = c // 2, c % 2
        sl = slice(g * DQ, (g + 1) * DQ)
        maps.append(
            {
                "x": np.ascontiguousarray(x[b]),
                "wq": np.ascontiguousarray(Wq[sl]),
                "wk": np.ascontiguousarray(Wk[sl]),
                "wv": np.ascontiguousarray(Wv[sl]),
            }
        )
    return maps


def _install_trace_hook():
    """Register the NTFF profile hook that trn_agent_boot skipped
    (antenv.axon_hooks module is absent in this image). Test-only."""
    import types

    if "antenv.axon_hooks" in sys.modules:
        return
    from trn_agent_boot.trn_boot import _ntff_profile_via_ctypes

    hook = _ntff_profile_via_ctypes("/opt/axon/libaxon_pjrt.so")
    m = types.ModuleType("antenv.axon_hooks")
    m.get_axon_ntff_profile_hook = lambda: hook
    m.set_axon_ntff_profile_hook = lambda h: None
    sys.modules["antenv.axon_hooks"] = m
    import antenv

    antenv.axon_hooks = m


def run(x, Wq, Wk, Wv, trace=False):
    """Run on 8 cores; returns (full output [4,2048,512], BassKernelResults)."""
    if trace:
        _install_trace_hook()
    nc = _get_nc()
    res = run_bass_kernel_spmd(nc, _in_maps(x, Wq, Wk, Wv), list(range(8)), trace=trace)
    out = np.empty((B, S, D), dtype=np.float32)
    for c in range(8):
        b, g = c // 2, c % 2
        out[b, :, g * DQ : (g + 1) * DQ] = res.results[c]["y"]
    return out, res


def kernel(x, Wq, Wk, Wv):
    out, _ = run(x, Wq, Wk, Wv)
    return out


if __name__ == "__main__":
    rng = np.random.default_rng(0)
    x = rng.standard_normal((B, S, D)).astype(np.float32)
    sc = 1.0 / np.sqrt(D)
    Wq = rng.uniform(-sc, sc, (D, D)).astype(np.float32)
    Wk = rng.uniform(-sc, sc, (D, D)).astype(np.float32)
    Wv = rng.uniform(-sc, sc, (D, D)).astype(np.float32)
    out = kernel(x, Wq, Wk, Wv)
    print("ran", out.shape, out.dtype)
